# revision 84
# baseline (speedup 1.0000x reference)
"""Trainium2 Bass kernel: masked contrastive loss, SPMD over 8 NeuronCores.

Math (reference: CustomContrastiveLoss):
  q = l2norm(logits.reshape(N,D)); k = l2norm(labels.reshape(N,D))
  sim = q @ k.T / TAU;  valid = pad_mask;  pos = (ad_i == ad_j) & valid_i & valid_j
  loss = mean_{valid rows} [ lse_valid(sim_row) - lse_pos(sim_row) ]
  (has_pos == valid because the diagonal is always a positive for valid rows)

Strategy (v4):
  * Host sorts the valid samples by ad value; invalid rows/cols drop out and
    each row's positives become one contiguous column range (rotated per core
    so the SPMD program is identical across cores, variation data-only).
  * Host normalizes, folds 1/TAU into queries, casts both operands to
    fp8e4m3, and emits PE-transposed k-tile-major layouts.  fp8 +
    MatmulPerfMode.DoubleRow contracts all of D=256 in ONE matmul
    (2x bf16 rate, half the DMA bytes).
  * |sim| <= ~8 after folding, so exp(sim - 20) needs no per-row max ->
    single pass.  ScalarE runs exp ACTIVATEs with accum_out giving the
    row-wise S_all partials; DVE does only the tiny masked band products
    (S_pos).  ScalarE is the pipeline wall, everything else overlaps.
  * Row remainder (V mod 1024 rows, when <=128) is handled as one
    column-sharded tail tile: every core exps [VR x ceil(V/8)] instead of
    a mostly-padding extra 128-row tile -- 8x less ScalarE waste.
  * Device ships raw S_all/S_pos partials plus an exp(-SHIFT) calibration
    value; host does ln / padding correction / mean on a [128, ~20] array.
"""

import os
import sys

for _p in ("/opt/trn_rl_repo", "/root/.axon_site/_ro/trn_rl_repo"):
    if os.path.isdir(_p) and _p not in sys.path:
        sys.path.append(_p)

import numpy as np
import ml_dtypes

import concourse.bass as bass
import concourse.mybir as mybir
import concourse.tile as tile
from concourse.bass_utils import run_bass_kernel_spmd

TAU = 0.05
INV_TAU = 1.0 / TAU
SHIFT = 20.0
# DVE fast-exp2: exp(x - SHIFT) ~= bf16frombits(round(x*FL + (FB - MAGIC)))
# via the fp32 round-to-nearest magic-add trick; FB folds the bf16 exponent
# bias, the -SHIFT shift, the mean-centering delta and the magic constant.
FL = 128.0 * 1.4426950408889634
FB = 12582912.0 + 128.0 * (127.0 - SHIFT * 1.4426950408889634 - 0.05744)
EPS = 1e-12
P = 128
D = 256
KC = D // P            # 2 k-tiles of 128 (DoubleRow consumes both at once)
NCORES = 8
BANK = 512             # PSUM bank width in fp32 (matmul out limit)
GRP = 1024             # max ACT group width (2 banks; 4 PSUM buffers)
DC = 1024              # label DMA chunk columns
NWARM = 2              # PE p-state warmup matmuls
F32 = mybir.dt.float32
BF16 = mybir.dt.bfloat16
FP8 = mybir.dt.float8e4
NPBF = ml_dtypes.bfloat16
NPF8 = ml_dtypes.float8_e4m3
AF = mybir.ActivationFunctionType
OP = mybir.AluOpType
DR = mybir.MatmulPerfMode.DoubleRow

# ---------------------------------------------------------------------------
# This walrus build rejects more than one sync-wait per instruction.  After
# Tile scheduling, hoist excess waits onto same-engine NOPs inserted right
# before the over-subscribed instruction (engine streams are sequential, so
# the waits still happen-before the instruction).
_MAXW = 1
_wsplit_n = [0]


def _split_excess_waits(nc):
    for f in nc.m.functions:
        for bb in f.blocks:
            insts = bb.instructions
            i = 0
            while i < len(insts):
                inst = insts[i]
                si = getattr(inst, "sync_info", None)
                if si is not None and si.on_wait and len(si.on_wait) > _MAXW:
                    waits = list(si.on_wait)
                    si.on_wait = waits[:_MAXW]
                    rest = waits[_MAXW:]
                    for j in range(0, len(rest), _MAXW):
                        _wsplit_n[0] += 1
                        nop = mybir.InstNoOp(
                            name=f"wsplit-{_wsplit_n[0]}", ins=[], outs=[]
                        )
                        nop.engine = inst.engine
                        nop.sync_info = mybir.SyncInfo(
                            on_wait=rest[j : j + _MAXW], on_update=[]
                        )
                        insts.insert(i, nop)
                        i += 1
                i += 1


def _roundup(a, b):
    return (a + b - 1) // b * b


def _chunks(V):
    """Label DMA chunk widths: six 512s first (fine-grained availability on
    the critical front), then 1024s; small remainder folded into the last."""
    cws = []
    rem = V
    while rem > 0:
        w = min(BANK, rem)
        cws.append(w)
        rem -= w
    if len(cws) >= 2 and cws[-1] < BANK // 2:
        cws[-2] += cws[-1]
        cws.pop()
    return cws


def _groups_for_tile(t, V):
    """ACT group widths for one row tile (each <= GRP, summing to V).
    Tile 0 splits its head so exp starts after a partial DMA chunk; a
    trailing sliver (<=128) is left in place -- it runs on DVE fast-exp."""
    gws = []
    rem = V
    caps = [BANK, BANK] if t == 0 else []
    while rem > 0:
        w = min(caps.pop(0) if caps else GRP, rem)
        gws.append(w)
        rem -= w
    # avoid mid-size slivers (128 < w < 512): rebalance the last two
    if len(gws) >= 2 and P < gws[-1] < BANK:
        s = gws[-2] + gws[-1]
        if s <= GRP:
            gws[-2:] = [s]
        else:
            h = (s // 2 + 15) // 16 * 16
            gws[-2:] = [h, s - h]
    return gws


def _segs(c0, gw, cws):
    """Split group [c0, c0+gw) into matmul segments that respect PSUM bank
    boundaries (512) and label chunk boundaries.  Yields (off, cc, coff, w)."""
    cbase = [0]
    for cw in cws:
        cbase.append(cbase[-1] + cw)
    off = 0
    while off < gw:
        c = c0 + off
        cc = 0
        while cbase[cc + 1] <= c:
            cc += 1
        w = min(gw - off, BANK - (off % BANK), cbase[cc + 1] - c)
        yield off, cc, c - cbase[cc], w
        off += w


def build_program(V, T, VR, VC, Wtot, win_starts, cws, groups):
    nch = len(cws)
    ns = sum(len(g) for g in groups)         # total S_all partial columns
    # res columns: [0,ns) accums; [ns,ns+T) spos; then tail sall, tail spos,
    # calibration exp(-SHIFT)
    c_ta, c_tp, c_cal = ns + T, ns + T + 1, ns + T + 2
    NC = ns + T + 3
    nc = bass.Bass("TRN2", target_bir_lowering=False, debug=False)
    # queries for row tiles 1.. plus the tail queries, one tensor/one DMA;
    # tile 0's queries ride at the head of chunk 0 (critical path)
    QW = (T - 1) * KC * P + KC * VR
    Q0 = KC * P
    qTd = nc.dram_tensor("qT", [P, max(QW, 1)], FP8, kind="ExternalInput")
    ysd = [nc.dram_tensor(f"ys{i}",
                          [P, KC * cws[i] + (Q0 if i == 0 else 0)],
                          FP8, kind="ExternalInput")
           for i in range(nch)]
    mskd = nc.dram_tensor("masks", [P, T * Wtot], BF16, kind="ExternalInput")
    if VR:
        ytld = nc.dram_tensor("ytail", [P, KC * VC], FP8, kind="ExternalInput")
        mtld = nc.dram_tensor("mtail", [VR, VC], BF16, kind="ExternalInput")
    outp = nc.dram_tensor("res", [P, NC], F32, kind="ExternalOutput")

    with tile.TileContext(nc) as tc:
        with (
            tc.tile_pool(name="singles", bufs=1) as singles,
            tc.tile_pool(name="tiny", bufs=2) as tiny,
            tc.tile_pool(name="est", bufs=2) as est_pool,
            tc.tile_pool(name="ubuf", bufs=2) as u_pool,
            tc.tile_pool(name="pmm", bufs=4, space="PSUM") as pmm,
        ):
            zro = singles.tile([P, 1], F32)
            nc.vector.memset(zro[:], 0.0)
            b_shift = singles.tile([P, 1], F32)
            nc.vector.memset(b_shift[:], -SHIFT)
            b_mag = singles.tile([P, 1], F32)
            nc.vector.memset(b_mag[:], FB)
            wz = singles.tile([P, KC, BANK], FP8)
            nc.vector.memset(wz[:], 0.0)

            qT = singles.tile([P, max(QW, 1)], FP8)
            ys = [singles.tile([P, KC * cws[i] + (Q0 if i == 0 else 0)],
                               FP8, name=f"ys{i}")
                  for i in range(nch)]
            masks = singles.tile([P, T * Wtot], BF16)
            res = singles.tile([P, NC], F32)
            nc.vector.memset(res[:], 0.0)
            if VR:
                ytl = singles.tile([P, KC, VC], FP8)
                mtl = singles.tile([VR, VC], BF16)
                etl = singles.tile([VR, VC], BF16)
                btl = singles.tile([VR, VC], BF16)

            qv = qT[:]
            y0v = ys[0][:]

            def q_ap(base, m):
                # [128, KC, m] view into the flat query tile
                return bass.AP(tensor=qv.tensor, offset=qv.offset + base,
                               ap=[qv.ap[0], [m, KC], [1, m]])

            def ys_ap(cc, coff, w):
                # [128, KC, w] view into chunk cc (chunk 0 is offset by the
                # packed tile-0 queries)
                t0 = ys[cc][:]
                off = Q0 if cc == 0 else 0
                return bass.AP(tensor=t0.tensor, offset=t0.offset + off + coff,
                               ap=[t0.ap[0], [cws[cc], KC], [1, w]])

            lhsT0 = bass.AP(tensor=y0v.tensor, offset=y0v.offset,
                            ap=[y0v.ap[0], [P, KC], [1, P]])

            # DMAs: every DGE queue is packet-rate limited, so spread the
            # label chunks round-robin over the gpsimd, scalar and sync
            # queues (scalar issues before its exp-table preload so the
            # transfers start immediately).
            # Queues run ~90 GB/s each and start at different times
            # (sync ~8.7us, scalar ~9.1, gpsimd ~10.1): spread the chunks
            # by their consumption deadlines.  The exp-table preload (also
            # the exp(-SHIFT) calibration output) slots between scalar's
            # early DMA issues so the table is ready when chunk 0 lands.
            def dma_ys(eng, i):
                if i < nch:
                    eng.dma_start(out=ys[i][:], in_=ysd[i].ap())

            dma_ys(nc.scalar, 0)
            dma_ys(nc.sync, 1)
            dma_ys(nc.scalar, 2)
            dma_ys(nc.sync, 3)
            nc.scalar.activation(out=res[:, c_cal:c_cal + 1], in_=zro[:],
                                 func=AF.Exp, bias=b_shift[:], scale=1.0)
            dma_ys(nc.scalar, 4)
            dma_ys(nc.sync, 5)
            dma_ys(nc.gpsimd, 6)
            for i in range(7, nch):
                (nc.gpsimd, nc.scalar, nc.sync)[i % 3].dma_start(
                    out=ys[i][:], in_=ysd[i].ap())
            nc.sync.dma_start(out=qT[:], in_=qTd.ap())
            nc.scalar.dma_start(out=masks[:], in_=mskd.ap())
            if VR:
                nc.scalar.dma_start(out=ytl[:], in_=ytld.ap())
                nc.gpsimd.dma_start(out=mtl[:], in_=mtld.ap())

            # PE warmup: keep the PE streaming while chunk 0 lands so the
            # p-state is ramped when real matmuls begin
            pw = pmm.tile([P, GRP], F32, tag="mm")
            for _ in range(NWARM):
                nc.tensor.matmul(pw[0:16, 0:BANK], wz[:, :, 0:16], wz[:],
                                 start=True, stop=True, perf_mode=DR)

            def full_tile(t, scol):
                est = est_pool.tile([P, V], BF16, name=f"est{t % 2}")
                lhsT = lhsT0 if t == 0 else q_ap((t - 1) * KC * P, P)
                w0 = win_starts[t]
                band_done = False
                c0 = 0
                ng = len(groups[t])
                dve_red = [2]          # per-tile budget of DVE row-sums
                for gi, gw in enumerate(groups[t]):
                    ps = pmm.tile([P, GRP], F32, tag="mm")
                    for off, cc, coff, w in _segs(c0, gw, cws):
                        nc.tensor.matmul(
                            ps[:, off:off + w], lhsT,
                            ys_ap(cc, coff, w),
                            start=True, stop=True, perf_mode=DR,
                        )
                    if gi == ng - 1 and band_done and ng > 1 and gw <= P:
                        # last group: DVE fast-exp2 + reduce (frees ScalarE)
                        u = u_pool.tile([P, gw], F32, name="ufast")
                        nc.vector.scalar_tensor_tensor(
                            out=u[:], in0=ps[:, :gw], scalar=FL,
                            in1=b_mag[:].broadcast_to((P, gw)),
                            op0=OP.mult, op1=OP.add)
                        bc = u[:].bitcast(BF16)
                        lo = bass.AP(tensor=bc.tensor, offset=bc.offset,
                                     ap=[bc.ap[0], [2, gw]])
                        nc.vector.tensor_reduce(
                            out=res[:, scol:scol + 1], in_=lo,
                            axis=mybir.AxisListType.X, op=OP.add)
                    elif band_done and dve_red[0] > 0 and gi < ng - 2:
                        # mid groups: exp on ACT without the accumulator
                        # read-out; the (mostly idle) DVE sums est instead
                        dve_red[0] -= 1
                        nc.scalar.activation(out=est[:, c0:c0 + gw],
                                             in_=ps[:, :gw], func=AF.Exp,
                                             bias=b_shift[:], scale=1.0)
                        nc.vector.tensor_reduce(
                            out=res[:, scol:scol + 1],
                            in_=est[:, c0:c0 + gw],
                            axis=mybir.AxisListType.X, op=OP.add)
                    else:
                        nc.scalar.activation(out=est[:, c0:c0 + gw],
                                             in_=ps[:, :gw], func=AF.Exp,
                                             bias=b_shift[:], scale=1.0,
                                             accum_out=res[:, scol:scol + 1])
                    scol += 1
                    c0 += gw
                    # band (S_pos) as soon as its window columns exist
                    if not band_done and c0 >= w0 + Wtot:
                        band_done = True
                        bscr = tiny.tile([P, Wtot], BF16)
                        nc.vector.tensor_mul(
                            out=bscr[:], in0=est[:, w0:w0 + Wtot],
                            in1=masks[:, t * Wtot:(t + 1) * Wtot])
                        nc.vector.tensor_reduce(
                            out=res[:, ns + t:ns + t + 1], in_=bscr[:],
                            axis=mybir.AxisListType.X, op=OP.add)
                assert band_done
                return scol

            def tail_tile():
                ps = pmm.tile([P, GRP], F32, tag="mm")
                qtl_ap = q_ap((T - 1) * KC * P, VR)
                off = 0
                while off < VC:
                    w = min(VC - off, BANK - (off % BANK))
                    nc.tensor.matmul(ps[0:VR, off:off + w], qtl_ap,
                                     ytl[:, :, off:off + w],
                                     start=True, stop=True, perf_mode=DR)
                    off += w
                # tail exp on DVE fast-exp2: ScalarE never touches the tail
                utl = u_pool.tile([VR, VC], F32, name="utail")
                nc.vector.scalar_tensor_tensor(
                    out=utl[:], in0=ps[0:VR, 0:VC], scalar=FL,
                    in1=b_mag[0:VR].broadcast_to((VR, VC)),
                    op0=OP.mult, op1=OP.add)
                bc = utl[:].bitcast(BF16)
                lo = bass.AP(tensor=bc.tensor, offset=bc.offset,
                             ap=[bc.ap[0], [2, VC]])
                nc.vector.tensor_reduce(out=res[0:VR, c_ta:c_ta + 1],
                                        in_=lo,
                                        axis=mybir.AxisListType.X, op=OP.add)
                nc.vector.tensor_mul(out=btl[:], in0=lo, in1=mtl[:])
                nc.vector.tensor_reduce(out=res[0:VR, c_tp:c_tp + 1],
                                        in_=btl[:],
                                        axis=mybir.AxisListType.X, op=OP.add)

            scol = full_tile(0, 0)
            if T > 1:
                scol = full_tile(1, scol)
            if VR:
                tail_tile()
            for t in range(2, T):
                scol = full_tile(t, scol)
            if VR and T <= 1:
                tail_tile()

            nc.scalar.dma_start(out=outp.ap(), in_=res[:])

    return nc


def plan(valid, ad):
    """Host-side sharding plan from the pad mask / ad ids (index math only)."""
    idx = np.nonzero(valid)[0]
    V = int(idx.size)
    if V == 0:
        return None
    order = idx[np.argsort(ad[idx], kind="stable")]
    ads = ad[order].astype(np.int64)
    W = int(np.bincount(ads).max())
    Wtot = min(_roundup(2 * W + P, 32), V)

    T = V // (NCORES * P)
    VR = V - T * NCORES * P
    if T == 0 or VR > P:
        # fallback: pad rows up to full tiles, no tail tile
        T = _roundup(V, NCORES * P) // (NCORES * P)
        VR = 0
    VC = (V + NCORES - 1) // NCORES if VR else 0

    R = T * P                                 # full-tile rows per core
    rotate = (R - P + Wtot <= V) and Wtot < V
    if rotate:
        win_starts = tuple(min(t * P, V - Wtot) for t in range(T))
    else:
        Wtot = V
        win_starts = (0,) * T
    cws = _chunks(V)
    groups = tuple(tuple(_groups_for_tile(t, V)) for t in range(T))
    return dict(V=V, R=R, T=T, VR=VR, VC=VC, W=W, Wtot=Wtot,
                win_starts=win_starts, rotate=rotate, order=order, ads=ads,
                cws=tuple(cws), groups=groups)


def host_prep(pl, x, y):
    """Normalize, fold 1/TAU into queries, cast fp8, build transposed
    k-tile-major layouts shared across cores."""
    order = pl["order"]
    V = pl["V"]
    xn = x[order]
    xnrm = np.sqrt(np.sum(xn * xn, axis=1, keepdims=True))
    qsc = (xn * (INV_TAU / np.maximum(xnrm, EPS))).astype(NPF8)   # [V, D]
    yn = y[order]
    ynrm = np.sqrt(np.sum(yn * yn, axis=1, keepdims=True))
    ksc = (yn / np.maximum(ynrm, EPS)).astype(NPF8)               # [V, D]
    # sorted transposed labels: ysT0[p, kc, v] = ksc[v, kc*P + p]
    ysT0 = np.ascontiguousarray(ksc.T.reshape(KC, P, V).transpose(1, 0, 2))
    return qsc, ysT0


def core_inputs(pl, qsc, ysT0, c):
    """Build core c's input arrays from the plan (host indexing only)."""
    V, R, W, T, VR, VC = (pl["V"], pl["R"], pl["W"], pl["T"], pl["VR"],
                          pl["VC"])
    ads = pl["ads"]
    g0 = c * R
    nv = max(0, min(R, V - g0))

    # queries: qT[p, (t*KC+i)*P + m] = qsc[g0 + t*P + m, i*P + p]; tile 0's
    # block is shipped at the head of label chunk 0 (critical path), the
    # rest plus the (shared) tail queries ride in qT as a single DMA
    qf = np.zeros((R, D), NPF8)
    if nv > 0:
        qf[:nv] = qsc[g0:g0 + nv]
    qTf = qf.reshape(T, P, KC, P).transpose(3, 0, 2, 1).reshape(P, T * KC * P)
    q0 = qTf[:, :KC * P]
    qT = qTf[:, KC * P:]
    if VR:
        r0 = T * NCORES * P
        qtl = (qsc[r0:r0 + VR].T.reshape(KC, P, VR).transpose(1, 0, 2)
               .reshape(P, KC * VR))
        qT = np.concatenate([qT, qtl], axis=1)
    if qT.shape[1] == 0:
        qT = np.zeros((P, 1), NPF8)
    qT = np.ascontiguousarray(qT)

    # labels: rotate sorted columns by (g0 - W) so each row tile's positives
    # land in its fixed window
    if pl["rotate"]:
        shift = (g0 - W) % V
        ysc = np.roll(ysT0, -shift, axis=2)
        adc_c = np.roll(ads, -shift)
    else:
        ysc = ysT0
        adc_c = ads
    cws = pl["cws"]
    inp = {"qT": qT}
    off = 0
    for i, cw in enumerate(cws):
        chunk = ysc[:, :, off:off + cw].reshape(P, KC * cw)
        if i == 0:
            chunk = np.concatenate([q0, chunk], axis=1)
        inp[f"ys{i}"] = np.ascontiguousarray(chunk)
        off += cw

    adr = np.ascontiguousarray(
        np.pad(ads[g0:g0 + nv].astype(np.float64), (0, R - nv),
               constant_values=-1.0).reshape(T, P).T)

    # band masks on host: masks[p, t*Wtot + j] = (adc[w0_t + j] == adr[p, t])
    Wtot = pl["Wtot"]
    masks = np.zeros((P, T, Wtot), NPBF)
    for t in range(T):
        w0 = pl["win_starts"][t]
        masks[:, t, :] = (adc_c[None, w0:w0 + Wtot] == adr[:, t:t + 1])
    inp["masks"] = np.ascontiguousarray(masks.reshape(P, T * Wtot))

    if VR:
        r0 = T * NCORES * P                   # first tail row (global)
        c0 = c * VC
        ncol = max(0, min(VC, V - c0))
        ytl = np.zeros((P, KC, VC), NPF8)
        ytl[:, :, :ncol] = ysT0[:, :, c0:c0 + ncol]
        inp["ytail"] = np.ascontiguousarray(ytl.reshape(P, KC * VC))
        mtl = np.zeros((VR, VC), NPBF)
        mtl[:, :ncol] = (ads[None, c0:c0 + ncol] == ads[r0:r0 + VR, None])
        inp["mtail"] = np.ascontiguousarray(mtl)
    return inp


_prog_cache = {}


def _get_program(pl):
    key = (pl["V"], pl["T"], pl["VR"], pl["VC"], pl["Wtot"],
           pl["win_starts"], pl["cws"], pl["groups"])
    if key not in _prog_cache:
        _prog_cache[key] = build_program(
            pl["V"], pl["T"], pl["VR"], pl["VC"], pl["Wtot"],
            pl["win_starts"], pl["cws"], pl["groups"]
        )
    return _prog_cache[key]


def kernel(logits, labels, pad_mask, ad_idxs, _want_results=False, **run_kwargs):
    x = np.ascontiguousarray(np.asarray(logits), dtype=np.float32).reshape(-1, D)
    y = np.ascontiguousarray(np.asarray(labels), dtype=np.float32).reshape(-1, D)
    valid = np.asarray(pad_mask).reshape(-1).astype(bool)
    ad = np.asarray(ad_idxs).reshape(-1).astype(np.int64)

    pl = plan(valid, ad)
    if pl is None:
        return np.float32(0.0)

    nc = _get_program(pl)
    # CoreSim chokes on the inserted NOPs, so split waits only for the HW path
    if not getattr(nc, "_waits_split", False):
        _split_excess_waits(nc)
        nc._waits_split = True
    qsc, ysT0 = host_prep(pl, x, y)
    in_maps = [core_inputs(pl, qsc, ysT0, c) for c in range(NCORES)]
    res = run_bass_kernel_spmd(nc, in_maps, core_ids=list(range(NCORES)),
                               **run_kwargs)

    V, T, R, VR, VC = pl["V"], pl["T"], pl["R"], pl["VR"], pl["VC"]
    ns = sum(len(g) for g in pl["groups"])
    c_ta, c_tp, c_cal = ns + T, ns + T + 1, ns + T + 2
    total = 0.0
    tail_sall = np.zeros(P)
    tail_spos = np.zeros(P)
    for c in range(NCORES):
        out = np.asarray(res.results[c]["res"], dtype=np.float64)  # [P, NC]
        sall = np.zeros((P, T))
        col = 0
        for t in range(T):
            for _ in pl["groups"][t]:
                sall[:, t] += out[:, col]
                col += 1
        spos = out[:, ns:ns + T]
        nv = max(0, min(R, V - c * R))
        if nv > 0:
            # row r of this core's shard: tile t = r // P, partition p = r % P
            dl = (np.log(np.maximum(sall, 1e-300))
                  - np.log(np.maximum(spos, 1e-300))).T.reshape(-1)
            total += float(np.sum(dl[:nv]))
        if VR:
            tail_sall += out[:, c_ta]
            tail_spos += out[:, c_tp]
    if VR:
        e0 = float(np.asarray(res.results[0]["res"], np.float64)[0, c_cal])
        npad = NCORES * VC - V
        ts_all = tail_sall[:VR] - npad * e0
        total += float(np.sum(np.log(np.maximum(ts_all, 1e-300))
                              - np.log(np.maximum(tail_spos[:VR], 1e-300))))
    loss = np.float32(total / V)
    if _want_results:
        return loss, res
    return loss


# revision 85
# speedup vs baseline: 1.0207x; 1.0207x over previous
"""Trainium2 Bass kernel: masked contrastive loss, SPMD over 8 NeuronCores.

Math (reference: CustomContrastiveLoss):
  q = l2norm(logits.reshape(N,D)); k = l2norm(labels.reshape(N,D))
  sim = q @ k.T / TAU;  valid = pad_mask;  pos = (ad_i == ad_j) & valid_i & valid_j
  loss = mean_{valid rows} [ lse_valid(sim_row) - lse_pos(sim_row) ]
  (has_pos == valid because the diagonal is always a positive for valid rows)

Strategy (v4):
  * Host sorts the valid samples by ad value; invalid rows/cols drop out and
    each row's positives become one contiguous column range (rotated per core
    so the SPMD program is identical across cores, variation data-only).
  * Host normalizes, folds 1/TAU into queries, casts both operands to
    fp8e4m3, and emits PE-transposed k-tile-major layouts.  fp8 +
    MatmulPerfMode.DoubleRow contracts all of D=256 in ONE matmul
    (2x bf16 rate, half the DMA bytes).
  * |sim| <= ~8 after folding, so exp(sim - 20) needs no per-row max ->
    single pass.  ScalarE runs exp ACTIVATEs with accum_out giving the
    row-wise S_all partials; DVE does only the tiny masked band products
    (S_pos).  ScalarE is the pipeline wall, everything else overlaps.
  * Row remainder (V mod 1024 rows, when <=128) is handled as one
    column-sharded tail tile: every core exps [VR x ceil(V/8)] instead of
    a mostly-padding extra 128-row tile -- 8x less ScalarE waste.
  * Device ships raw S_all/S_pos partials plus an exp(-SHIFT) calibration
    value; host does ln / padding correction / mean on a [128, ~20] array.
"""

import os
import sys

for _p in ("/opt/trn_rl_repo", "/root/.axon_site/_ro/trn_rl_repo"):
    if os.path.isdir(_p) and _p not in sys.path:
        sys.path.append(_p)

import numpy as np
import ml_dtypes

import concourse.bass as bass
import concourse.mybir as mybir
import concourse.tile as tile
from concourse.bass_utils import run_bass_kernel_spmd

TAU = 0.05
INV_TAU = 1.0 / TAU
SHIFT = 20.0
# DVE fast-exp2: exp(x - SHIFT) ~= bf16frombits(round(x*FL + (FB - MAGIC)))
# via the fp32 round-to-nearest magic-add trick; FB folds the bf16 exponent
# bias, the -SHIFT shift, the mean-centering delta and the magic constant.
FL = 128.0 * 1.4426950408889634
FB = 12582912.0 + 128.0 * (127.0 - SHIFT * 1.4426950408889634 - 0.05744)
EPS = 1e-12
P = 128
D = 256
KC = D // P            # 2 k-tiles of 128 (DoubleRow consumes both at once)
NCORES = 8
BANK = 512             # PSUM bank width in fp32 (matmul out limit)
GRP = 1024             # max ACT group width (2 banks; 4 PSUM buffers)
DC = 1024              # label DMA chunk columns
NWARM = 4              # PE p-state warmup matmuls
F32 = mybir.dt.float32
BF16 = mybir.dt.bfloat16
FP8 = mybir.dt.float8e4
NPBF = ml_dtypes.bfloat16
NPF8 = ml_dtypes.float8_e4m3
AF = mybir.ActivationFunctionType
OP = mybir.AluOpType
DR = mybir.MatmulPerfMode.DoubleRow

# ---------------------------------------------------------------------------
# This walrus build rejects more than one sync-wait per instruction.  After
# Tile scheduling, hoist excess waits onto same-engine NOPs inserted right
# before the over-subscribed instruction (engine streams are sequential, so
# the waits still happen-before the instruction).
_MAXW = 1
_wsplit_n = [0]


def _split_excess_waits(nc):
    for f in nc.m.functions:
        for bb in f.blocks:
            insts = bb.instructions
            i = 0
            while i < len(insts):
                inst = insts[i]
                si = getattr(inst, "sync_info", None)
                if si is not None and si.on_wait and len(si.on_wait) > _MAXW:
                    waits = list(si.on_wait)
                    si.on_wait = waits[:_MAXW]
                    rest = waits[_MAXW:]
                    for j in range(0, len(rest), _MAXW):
                        _wsplit_n[0] += 1
                        nop = mybir.InstNoOp(
                            name=f"wsplit-{_wsplit_n[0]}", ins=[], outs=[]
                        )
                        nop.engine = inst.engine
                        nop.sync_info = mybir.SyncInfo(
                            on_wait=rest[j : j + _MAXW], on_update=[]
                        )
                        insts.insert(i, nop)
                        i += 1
                i += 1


def _roundup(a, b):
    return (a + b - 1) // b * b


def _chunks(V):
    """Label DMA chunk widths: six 512s first (fine-grained availability on
    the critical front), then 1024s; small remainder folded into the last."""
    cws = []
    rem = V
    while rem > 0:
        w = min(BANK, rem)
        cws.append(w)
        rem -= w
    if len(cws) >= 2 and cws[-1] < BANK // 2:
        cws[-2] += cws[-1]
        cws.pop()
    return cws


def _groups_for_tile(t, V):
    """ACT group widths for one row tile (each <= GRP, summing to V).
    Tile 0 splits its head so exp starts after a partial DMA chunk; a
    trailing sliver (<=128) is left in place -- it runs on DVE fast-exp."""
    gws = []
    rem = V
    caps = [BANK, BANK] if t == 0 else []
    while rem > 0:
        w = min(caps.pop(0) if caps else GRP, rem)
        gws.append(w)
        rem -= w
    # avoid mid-size slivers (128 < w < 512): rebalance the last two
    if len(gws) >= 2 and P < gws[-1] < BANK:
        s = gws[-2] + gws[-1]
        if s <= GRP:
            gws[-2:] = [s]
        else:
            h = (s // 2 + 15) // 16 * 16
            gws[-2:] = [h, s - h]
    return gws


def _segs(c0, gw, cws):
    """Split group [c0, c0+gw) into matmul segments that respect PSUM bank
    boundaries (512) and label chunk boundaries.  Yields (off, cc, coff, w)."""
    cbase = [0]
    for cw in cws:
        cbase.append(cbase[-1] + cw)
    off = 0
    while off < gw:
        c = c0 + off
        cc = 0
        while cbase[cc + 1] <= c:
            cc += 1
        w = min(gw - off, BANK - (off % BANK), cbase[cc + 1] - c)
        yield off, cc, c - cbase[cc], w
        off += w


def build_program(V, T, VR, VC, Wtot, win_starts, cws, groups):
    nch = len(cws)
    ns = sum(len(g) for g in groups)         # total S_all partial columns
    # res columns: [0,ns) accums; [ns,ns+T) spos; then tail sall, tail spos,
    # calibration exp(-SHIFT)
    c_ta, c_tp, c_cal = ns + T, ns + T + 1, ns + T + 2
    NC = ns + T + 3
    nc = bass.Bass("TRN2", target_bir_lowering=False, debug=False)
    # queries for row tiles 1.. plus the tail queries, one tensor/one DMA;
    # tile 0's queries ride at the head of chunk 0 (critical path)
    QW = (T - 1) * KC * P + KC * VR
    Q0 = KC * P
    qTd = nc.dram_tensor("qT", [P, max(QW, 1)], FP8, kind="ExternalInput")
    ysd = [nc.dram_tensor(f"ys{i}",
                          [P, KC * cws[i] + (Q0 if i == 0 else 0)],
                          FP8, kind="ExternalInput")
           for i in range(nch)]
    mskd = nc.dram_tensor("masks", [P, T * Wtot], BF16, kind="ExternalInput")
    if VR:
        ytld = nc.dram_tensor("ytail", [P, KC * VC], FP8, kind="ExternalInput")
        mtld = nc.dram_tensor("mtail", [VR, VC], BF16, kind="ExternalInput")
    outp = nc.dram_tensor("res", [P, NC], F32, kind="ExternalOutput")

    with tile.TileContext(nc) as tc:
        with (
            tc.tile_pool(name="singles", bufs=1) as singles,
            tc.tile_pool(name="tiny", bufs=2) as tiny,
            tc.tile_pool(name="est", bufs=2) as est_pool,
            tc.tile_pool(name="ubuf", bufs=2) as u_pool,
            tc.tile_pool(name="pmm", bufs=4, space="PSUM") as pmm,
        ):
            zro = singles.tile([P, 1], F32)
            nc.vector.memset(zro[:], 0.0)
            b_shift = singles.tile([P, 1], F32)
            nc.vector.memset(b_shift[:], -SHIFT)
            b_mag = singles.tile([P, 1], F32)
            nc.vector.memset(b_mag[:], FB)
            wz = singles.tile([P, KC, BANK], FP8)
            nc.vector.memset(wz[:], 0.0)

            qT = singles.tile([P, max(QW, 1)], FP8)
            ys = [singles.tile([P, KC * cws[i] + (Q0 if i == 0 else 0)],
                               FP8, name=f"ys{i}")
                  for i in range(nch)]
            masks = singles.tile([P, T * Wtot], BF16)
            res = singles.tile([P, NC], F32)
            nc.vector.memset(res[:], 0.0)
            if VR:
                ytl = singles.tile([P, KC, VC], FP8)
                mtl = singles.tile([VR, VC], BF16)
                etl = singles.tile([VR, VC], BF16)
                btl = singles.tile([VR, VC], BF16)

            qv = qT[:]
            y0v = ys[0][:]

            def q_ap(base, m):
                # [128, KC, m] view into the flat query tile
                return bass.AP(tensor=qv.tensor, offset=qv.offset + base,
                               ap=[qv.ap[0], [m, KC], [1, m]])

            def ys_ap(cc, coff, w):
                # [128, KC, w] view into chunk cc (chunk 0 is offset by the
                # packed tile-0 queries)
                t0 = ys[cc][:]
                off = Q0 if cc == 0 else 0
                return bass.AP(tensor=t0.tensor, offset=t0.offset + off + coff,
                               ap=[t0.ap[0], [cws[cc], KC], [1, w]])

            lhsT0 = bass.AP(tensor=y0v.tensor, offset=y0v.offset,
                            ap=[y0v.ap[0], [P, KC], [1, P]])

            # DMAs: every DGE queue is packet-rate limited, so spread the
            # label chunks round-robin over the gpsimd, scalar and sync
            # queues (scalar issues before its exp-table preload so the
            # transfers start immediately).
            # Queues run ~90 GB/s each and start at different times
            # (sync ~8.7us, scalar ~9.1, gpsimd ~10.1): spread the chunks
            # by their consumption deadlines.  The exp-table preload (also
            # the exp(-SHIFT) calibration output) slots between scalar's
            # early DMA issues so the table is ready when chunk 0 lands.
            def dma_ys(eng, i):
                if i < nch:
                    eng.dma_start(out=ys[i][:], in_=ysd[i].ap())

            dma_ys(nc.scalar, 0)
            dma_ys(nc.sync, 1)
            dma_ys(nc.scalar, 2)
            dma_ys(nc.sync, 3)
            nc.scalar.activation(out=res[:, c_cal:c_cal + 1], in_=zro[:],
                                 func=AF.Exp, bias=b_shift[:], scale=1.0)
            dma_ys(nc.scalar, 4)
            dma_ys(nc.sync, 5)
            dma_ys(nc.gpsimd, 6)
            for i in range(7, nch):
                (nc.gpsimd, nc.scalar, nc.sync)[i % 3].dma_start(
                    out=ys[i][:], in_=ysd[i].ap())
            nc.sync.dma_start(out=qT[:], in_=qTd.ap())
            nc.scalar.dma_start(out=masks[:], in_=mskd.ap())
            if VR:
                nc.scalar.dma_start(out=ytl[:], in_=ytld.ap())
                nc.gpsimd.dma_start(out=mtl[:], in_=mtld.ap())

            # PE warmup: keep the PE streaming while chunk 0 lands so the
            # p-state is ramped when real matmuls begin
            pw = pmm.tile([P, GRP], F32, tag="mm")
            for _ in range(NWARM):
                nc.tensor.matmul(pw[0:16, 0:BANK], wz[:, :, 0:16], wz[:],
                                 start=True, stop=True, perf_mode=DR)

            def full_tile(t, scol):
                est = est_pool.tile([P, V], BF16, name=f"est{t % 2}")
                lhsT = lhsT0 if t == 0 else q_ap((t - 1) * KC * P, P)
                w0 = win_starts[t]
                band_done = False
                c0 = 0
                ng = len(groups[t])
                dve_red = [2]          # per-tile budget of DVE row-sums
                for gi, gw in enumerate(groups[t]):
                    ps = pmm.tile([P, GRP], F32, tag="mm")
                    for off, cc, coff, w in _segs(c0, gw, cws):
                        nc.tensor.matmul(
                            ps[:, off:off + w], lhsT,
                            ys_ap(cc, coff, w),
                            start=True, stop=True, perf_mode=DR,
                        )
                    if gi == ng - 1 and band_done and ng > 1 and gw <= P:
                        # last group: DVE fast-exp2 + reduce (frees ScalarE)
                        u = u_pool.tile([P, gw], F32, name="ufast")
                        nc.vector.scalar_tensor_tensor(
                            out=u[:], in0=ps[:, :gw], scalar=FL,
                            in1=b_mag[:].broadcast_to((P, gw)),
                            op0=OP.mult, op1=OP.add)
                        bc = u[:].bitcast(BF16)
                        lo = bass.AP(tensor=bc.tensor, offset=bc.offset,
                                     ap=[bc.ap[0], [2, gw]])
                        nc.vector.tensor_reduce(
                            out=res[:, scol:scol + 1], in_=lo,
                            axis=mybir.AxisListType.X, op=OP.add)
                    elif band_done and dve_red[0] > 0 and gi < ng - 2:
                        # mid groups: exp on ACT without the accumulator
                        # read-out; the (mostly idle) DVE sums est instead
                        dve_red[0] -= 1
                        nc.scalar.activation(out=est[:, c0:c0 + gw],
                                             in_=ps[:, :gw], func=AF.Exp,
                                             bias=b_shift[:], scale=1.0)
                        nc.vector.tensor_reduce(
                            out=res[:, scol:scol + 1],
                            in_=est[:, c0:c0 + gw],
                            axis=mybir.AxisListType.X, op=OP.add)
                    else:
                        nc.scalar.activation(out=est[:, c0:c0 + gw],
                                             in_=ps[:, :gw], func=AF.Exp,
                                             bias=b_shift[:], scale=1.0,
                                             accum_out=res[:, scol:scol + 1])
                    scol += 1
                    c0 += gw
                    # band (S_pos) as soon as its window columns exist
                    if not band_done and c0 >= w0 + Wtot:
                        band_done = True
                        bscr = tiny.tile([P, Wtot], BF16)
                        nc.vector.tensor_mul(
                            out=bscr[:], in0=est[:, w0:w0 + Wtot],
                            in1=masks[:, t * Wtot:(t + 1) * Wtot])
                        nc.vector.tensor_reduce(
                            out=res[:, ns + t:ns + t + 1], in_=bscr[:],
                            axis=mybir.AxisListType.X, op=OP.add)
                assert band_done
                return scol

            def tail_tile():
                ps = pmm.tile([P, GRP], F32, tag="mm")
                qtl_ap = q_ap((T - 1) * KC * P, VR)
                off = 0
                while off < VC:
                    w = min(VC - off, BANK - (off % BANK))
                    nc.tensor.matmul(ps[0:VR, off:off + w], qtl_ap,
                                     ytl[:, :, off:off + w],
                                     start=True, stop=True, perf_mode=DR)
                    off += w
                # tail exp on DVE fast-exp2: ScalarE never touches the tail
                utl = u_pool.tile([VR, VC], F32, name="utail")
                nc.vector.scalar_tensor_tensor(
                    out=utl[:], in0=ps[0:VR, 0:VC], scalar=FL,
                    in1=b_mag[0:VR].broadcast_to((VR, VC)),
                    op0=OP.mult, op1=OP.add)
                bc = utl[:].bitcast(BF16)
                lo = bass.AP(tensor=bc.tensor, offset=bc.offset,
                             ap=[bc.ap[0], [2, VC]])
                nc.vector.tensor_reduce(out=res[0:VR, c_ta:c_ta + 1],
                                        in_=lo,
                                        axis=mybir.AxisListType.X, op=OP.add)
                nc.vector.tensor_mul(out=btl[:], in0=lo, in1=mtl[:])
                nc.vector.tensor_reduce(out=res[0:VR, c_tp:c_tp + 1],
                                        in_=btl[:],
                                        axis=mybir.AxisListType.X, op=OP.add)

            scol = full_tile(0, 0)
            if T > 1:
                scol = full_tile(1, scol)
            if VR:
                tail_tile()
            for t in range(2, T):
                scol = full_tile(t, scol)
            if VR and T <= 1:
                tail_tile()

            nc.scalar.dma_start(out=outp.ap(), in_=res[:])

    return nc


def plan(valid, ad):
    """Host-side sharding plan from the pad mask / ad ids (index math only)."""
    idx = np.nonzero(valid)[0]
    V = int(idx.size)
    if V == 0:
        return None
    order = idx[np.argsort(ad[idx], kind="stable")]
    ads = ad[order].astype(np.int64)
    W = int(np.bincount(ads).max())
    Wtot = min(_roundup(2 * W + P, 32), V)

    T = V // (NCORES * P)
    VR = V - T * NCORES * P
    if T == 0 or VR > P:
        # fallback: pad rows up to full tiles, no tail tile
        T = _roundup(V, NCORES * P) // (NCORES * P)
        VR = 0
    VC = (V + NCORES - 1) // NCORES if VR else 0

    R = T * P                                 # full-tile rows per core
    rotate = (R - P + Wtot <= V) and Wtot < V
    if rotate:
        win_starts = tuple(min(t * P, V - Wtot) for t in range(T))
    else:
        Wtot = V
        win_starts = (0,) * T
    cws = _chunks(V)
    groups = tuple(tuple(_groups_for_tile(t, V)) for t in range(T))
    return dict(V=V, R=R, T=T, VR=VR, VC=VC, W=W, Wtot=Wtot,
                win_starts=win_starts, rotate=rotate, order=order, ads=ads,
                cws=tuple(cws), groups=groups)


def host_prep(pl, x, y):
    """Normalize, fold 1/TAU into queries, cast fp8, build transposed
    k-tile-major layouts shared across cores."""
    order = pl["order"]
    V = pl["V"]
    xn = x[order]
    xnrm = np.sqrt(np.sum(xn * xn, axis=1, keepdims=True))
    qsc = (xn * (INV_TAU / np.maximum(xnrm, EPS))).astype(NPF8)   # [V, D]
    yn = y[order]
    ynrm = np.sqrt(np.sum(yn * yn, axis=1, keepdims=True))
    ksc = (yn / np.maximum(ynrm, EPS)).astype(NPF8)               # [V, D]
    # sorted transposed labels: ysT0[p, kc, v] = ksc[v, kc*P + p]
    ysT0 = np.ascontiguousarray(ksc.T.reshape(KC, P, V).transpose(1, 0, 2))
    return qsc, ysT0


def core_inputs(pl, qsc, ysT0, c):
    """Build core c's input arrays from the plan (host indexing only)."""
    V, R, W, T, VR, VC = (pl["V"], pl["R"], pl["W"], pl["T"], pl["VR"],
                          pl["VC"])
    ads = pl["ads"]
    g0 = c * R
    nv = max(0, min(R, V - g0))

    # queries: qT[p, (t*KC+i)*P + m] = qsc[g0 + t*P + m, i*P + p]; tile 0's
    # block is shipped at the head of label chunk 0 (critical path), the
    # rest plus the (shared) tail queries ride in qT as a single DMA
    qf = np.zeros((R, D), NPF8)
    if nv > 0:
        qf[:nv] = qsc[g0:g0 + nv]
    qTf = qf.reshape(T, P, KC, P).transpose(3, 0, 2, 1).reshape(P, T * KC * P)
    q0 = qTf[:, :KC * P]
    qT = qTf[:, KC * P:]
    if VR:
        r0 = T * NCORES * P
        qtl = (qsc[r0:r0 + VR].T.reshape(KC, P, VR).transpose(1, 0, 2)
               .reshape(P, KC * VR))
        qT = np.concatenate([qT, qtl], axis=1)
    if qT.shape[1] == 0:
        qT = np.zeros((P, 1), NPF8)
    qT = np.ascontiguousarray(qT)

    # labels: rotate sorted columns by (g0 - W) so each row tile's positives
    # land in its fixed window
    if pl["rotate"]:
        shift = (g0 - W) % V
        ysc = np.roll(ysT0, -shift, axis=2)
        adc_c = np.roll(ads, -shift)
    else:
        ysc = ysT0
        adc_c = ads
    cws = pl["cws"]
    inp = {"qT": qT}
    off = 0
    for i, cw in enumerate(cws):
        chunk = ysc[:, :, off:off + cw].reshape(P, KC * cw)
        if i == 0:
            chunk = np.concatenate([q0, chunk], axis=1)
        inp[f"ys{i}"] = np.ascontiguousarray(chunk)
        off += cw

    adr = np.ascontiguousarray(
        np.pad(ads[g0:g0 + nv].astype(np.float64), (0, R - nv),
               constant_values=-1.0).reshape(T, P).T)

    # band masks on host: masks[p, t*Wtot + j] = (adc[w0_t + j] == adr[p, t])
    Wtot = pl["Wtot"]
    masks = np.zeros((P, T, Wtot), NPBF)
    for t in range(T):
        w0 = pl["win_starts"][t]
        masks[:, t, :] = (adc_c[None, w0:w0 + Wtot] == adr[:, t:t + 1])
    inp["masks"] = np.ascontiguousarray(masks.reshape(P, T * Wtot))

    if VR:
        r0 = T * NCORES * P                   # first tail row (global)
        c0 = c * VC
        ncol = max(0, min(VC, V - c0))
        ytl = np.zeros((P, KC, VC), NPF8)
        ytl[:, :, :ncol] = ysT0[:, :, c0:c0 + ncol]
        inp["ytail"] = np.ascontiguousarray(ytl.reshape(P, KC * VC))
        mtl = np.zeros((VR, VC), NPBF)
        mtl[:, :ncol] = (ads[None, c0:c0 + ncol] == ads[r0:r0 + VR, None])
        inp["mtail"] = np.ascontiguousarray(mtl)
    return inp


_prog_cache = {}


def _get_program(pl):
    key = (pl["V"], pl["T"], pl["VR"], pl["VC"], pl["Wtot"],
           pl["win_starts"], pl["cws"], pl["groups"])
    if key not in _prog_cache:
        _prog_cache[key] = build_program(
            pl["V"], pl["T"], pl["VR"], pl["VC"], pl["Wtot"],
            pl["win_starts"], pl["cws"], pl["groups"]
        )
    return _prog_cache[key]


def kernel(logits, labels, pad_mask, ad_idxs, _want_results=False, **run_kwargs):
    x = np.ascontiguousarray(np.asarray(logits), dtype=np.float32).reshape(-1, D)
    y = np.ascontiguousarray(np.asarray(labels), dtype=np.float32).reshape(-1, D)
    valid = np.asarray(pad_mask).reshape(-1).astype(bool)
    ad = np.asarray(ad_idxs).reshape(-1).astype(np.int64)

    pl = plan(valid, ad)
    if pl is None:
        return np.float32(0.0)

    nc = _get_program(pl)
    # CoreSim chokes on the inserted NOPs, so split waits only for the HW path
    if not getattr(nc, "_waits_split", False):
        _split_excess_waits(nc)
        nc._waits_split = True
    qsc, ysT0 = host_prep(pl, x, y)
    in_maps = [core_inputs(pl, qsc, ysT0, c) for c in range(NCORES)]
    res = run_bass_kernel_spmd(nc, in_maps, core_ids=list(range(NCORES)),
                               **run_kwargs)

    V, T, R, VR, VC = pl["V"], pl["T"], pl["R"], pl["VR"], pl["VC"]
    ns = sum(len(g) for g in pl["groups"])
    c_ta, c_tp, c_cal = ns + T, ns + T + 1, ns + T + 2
    total = 0.0
    tail_sall = np.zeros(P)
    tail_spos = np.zeros(P)
    for c in range(NCORES):
        out = np.asarray(res.results[c]["res"], dtype=np.float64)  # [P, NC]
        sall = np.zeros((P, T))
        col = 0
        for t in range(T):
            for _ in pl["groups"][t]:
                sall[:, t] += out[:, col]
                col += 1
        spos = out[:, ns:ns + T]
        nv = max(0, min(R, V - c * R))
        if nv > 0:
            # row r of this core's shard: tile t = r // P, partition p = r % P
            dl = (np.log(np.maximum(sall, 1e-300))
                  - np.log(np.maximum(spos, 1e-300))).T.reshape(-1)
            total += float(np.sum(dl[:nv]))
        if VR:
            tail_sall += out[:, c_ta]
            tail_spos += out[:, c_tp]
    if VR:
        e0 = float(np.asarray(res.results[0]["res"], np.float64)[0, c_cal])
        npad = NCORES * VC - V
        ts_all = tail_sall[:VR] - npad * e0
        total += float(np.sum(np.log(np.maximum(ts_all, 1e-300))
                              - np.log(np.maximum(tail_spos[:VR], 1e-300))))
    loss = np.float32(total / V)
    if _want_results:
        return loss, res
    return loss
